# revision 1
# baseline (speedup 1.0000x reference)
"""Trainium2 Bass kernel for nn_DifferentiableParallelBeamRadon.

Reference op: parallel-beam Radon transform of image [4,1,256,256] over 180
angles -> sinogram [4,1,180,256] (torch-style affine_grid/grid_sample bilinear
sampling with zeros padding, summed over rotated rows, scaled by 2/255).

Strategy
--------
Geometry is input-independent, so at import we precompute, per angle:
  * binning axis (by image row when |tan theta|<=1, else by image column on
    the transposed image),
  * for each (bin p, detector j) a contiguous <=4-cell tap window base
    XIDX[p,j] along the other axis,
  * coefficient planes C[r,p,j] (r < R_a <= 4) holding the bilinear weights
    of every (source-row i, tap) pair, binned in float64, with the
    reference's 2/255 scale folded in.

Then  sino[b,j] = sum_p sum_r C[r,p,j] * IMG_axis[b, p, XIDX[p,j]+r].

At call time the host gathers the tap windows G (pure data layout; all
arithmetic runs on device) and ships G + C to the 8 NeuronCores, angles
sharded 8 ways (R-sorted round-robin so SPMD slot sizes match).  Each core,
per angle, computes P = C (*) G with one VectorE tensor_tensor (fp16, 2x
mode, C broadcast along the batch axis) and reduces over the 128 partitions
(bins) with ones-vector matmuls on TensorE, accumulating all (r, bin-half)
planes of one angle in PSUM; ScalarE drains PSUM into a staging row that is
DMA'd out once at the end.
"""

import os

import numpy as np

IMAGE_SIZE = 256
NUM_ANGLES = 180
NUM_DET = 256
BATCH = 4
N_CORES = 8
R_MAX = 4
PAD = 4
WPAD = IMAGE_SIZE + 2 * PAD  # 264

USE_F32 = bool(int(os.environ.get("RADON_F32", "0")))
_DT_NP = np.float32 if USE_F32 else np.float16

NSLOT = (NUM_ANGLES + N_CORES - 1) // N_CORES  # 23


# ----------------------------------------------------------------------------
# geometry precompute (input independent, cached at import)
# ----------------------------------------------------------------------------

def _angle_tables(a_idx: int):
    """Return (axis, xidx int32 [256,256], C float64 [R_MAX,256,256])."""
    N = IMAGE_SIZE
    angles = np.linspace(0.0, 180.0, NUM_ANGLES + 1, dtype=np.float32)[:-1]
    ang = np.deg2rad(angles[a_idx], dtype=np.float32)
    cos = np.cos(ang, dtype=np.float32)
    sin = np.sin(ang, dtype=np.float32)

    j = np.arange(N, dtype=np.float32)
    xs = ((2.0 * j + 1.0) / np.float32(N) - 1.0).astype(np.float32)
    ys = xs.copy()

    gx = (cos * xs[None, :] + sin * ys[:, None]).astype(np.float32)
    gy = (-sin * xs[None, :] + cos * ys[:, None]).astype(np.float32)
    ix = (((gx + 1.0) * np.float32(N) - 1.0) * np.float32(0.5)).astype(np.float32)
    iy = (((gy + 1.0) * np.float32(N) - 1.0) * np.float32(0.5)).astype(np.float32)

    x0 = np.floor(ix)
    y0 = np.floor(iy)
    wx1 = (ix - x0).astype(np.float64)
    wy1 = (iy - y0).astype(np.float64)
    wx0 = 1.0 - wx1
    wy0 = 1.0 - wy1
    x0 = x0.astype(np.int64)
    y0 = y0.astype(np.int64)

    bin_by_row = abs(float(sin)) <= abs(float(cos))

    taps = [
        (y0, x0, wy0 * wx0),
        (y0, x0 + 1, wy0 * wx1),
        (y0 + 1, x0, wy1 * wx0),
        (y0 + 1, x0 + 1, wy1 * wx1),
    ]

    INF = 1 << 20
    qmin = np.full((N, N), INF, dtype=np.int64)
    qmax = np.full((N, N), -INF, dtype=np.int64)
    jj = np.broadcast_to(np.arange(N)[None, :], (N, N))
    binned = []
    for (rr, cc, w) in taps:
        valid = (rr >= 0) & (rr < N) & (cc >= 0) & (cc < N)
        bp, q = (rr, cc) if bin_by_row else (cc, rr)
        m = valid & (w > 0)
        binned.append((bp, q, w, m))
        np.minimum.at(qmin, (bp[m], jj[m]), q[m])
        np.maximum.at(qmax, (bp[m], jj[m]), q[m])

    width = np.where(qmin <= qmax, qmax - qmin + 1, 0)
    assert width.max() <= R_MAX, f"angle {a_idx}: window {width.max()}"
    qbase = np.where(qmin == INF, 0, qmin)

    C = np.zeros((R_MAX, N, N), dtype=np.float64)
    for (bp, q, w, m) in binned:
        r = q[m] - qbase[bp[m], jj[m]]
        np.add.at(C, (r, bp[m], jj[m]), w[m])

    C *= 2.0 / (IMAGE_SIZE - 1)
    return (0 if bin_by_row else 1), qbase.astype(np.int32), C


_TABLES = None


def _get_tables():
    """Cached geometry:
    SLOT_ANGLE[s][k] -> angle index or -1, R_SLOT[s], and per angle:
    AXIS[a], FIDX[a] (int32 [R_a,256,256] flat gather idx), CDEV[a]
    ([128, R_a*2*256] coefficient tile, device layout, dtype _DT_NP).
    """
    global _TABLES
    if _TABLES is not None:
        return _TABLES

    axes = np.zeros(NUM_ANGLES, dtype=np.int64)
    r_eff = np.zeros(NUM_ANGLES, dtype=np.int64)
    fidx_all = []
    cdev_all = []
    for a in range(NUM_ANGLES):
        axis, xidx, C = _angle_tables(a)
        axes[a] = axis
        nz = [r for r in range(R_MAX) if np.abs(C[r]).max() > 0]
        Ra = (max(nz) + 1) if nz else 1
        r_eff[a] = Ra
        rr = np.arange(Ra)[:, None, None]
        pp = np.arange(IMAGE_SIZE)[None, :, None]
        f = pp * WPAD + (xidx[None] + rr + PAD)
        assert f.min() >= 0 and f.max() < IMAGE_SIZE * WPAD
        fidx_all.append(f.astype(np.int32))
        # device layout [pl 128, (r Ra, h 2, j 256)]
        cl = C[:Ra].reshape(Ra, 2, 128, NUM_DET).transpose(2, 0, 1, 3)
        cdev_all.append(
            np.ascontiguousarray(cl.reshape(128, -1).astype(_DT_NP))
        )

    # R-sorted (desc) round-robin slot assignment so SPMD slot sizes match
    order = np.argsort(-r_eff, kind="stable")
    slot_angle = np.full((NSLOT, N_CORES), -1, dtype=np.int64)
    for i, a in enumerate(order):
        slot_angle[i // N_CORES, i % N_CORES] = a
    r_slot = np.array(
        [max(1, max(r_eff[a] for a in row if a >= 0)) for row in slot_angle]
    )

    _TABLES = (axes, r_eff, fidx_all, cdev_all, slot_angle, r_slot)
    return _TABLES


# ----------------------------------------------------------------------------
# bass program (built once, cached)
# ----------------------------------------------------------------------------

_PROG = {}


def _build_program(loop: int | None = None):
    """Build (and cache) the Bass program.  loop>1 wraps the body in a
    device-side For_i — timing-measurement only."""
    if loop is None:
        loop = int(os.environ.get("RADON_LOOP", "0"))
    key = loop
    if key in _PROG:
        return _PROG[key]
    import concourse.bacc as bacc
    import concourse.mybir as mybir
    from concourse.tile import TileContext

    _, _, _, _, _, r_slot = _get_tables()

    dt_data = mybir.dt.float32 if USE_F32 else mybir.dt.float16

    REPEAT = int(os.environ.get("RADON_REPEAT", "1"))
    LOOP = loop

    g_sizes = [int(r) * 2 * BATCH * NUM_DET for r in r_slot]
    c_sizes = [int(r) * 2 * NUM_DET for r in r_slot]
    # G and C packed adjacently per slot -> one DMA per slot
    gc_sizes = [g + c for g, c in zip(g_sizes, c_sizes)]
    gc_off = np.concatenate([[0], np.cumsum(gc_sizes)])
    TOT = int(gc_off[-1])
    GCMAX = max(gc_sizes)
    # f32 debug mode: no slot pairing (SBUF budget)
    PAIR = 1 if USE_F32 else 2
    PAIRMAX = max(
        sum(gc_sizes[t] for t in range(s, min(s + PAIR, NSLOT)))
        for s in range(0, NSLOT, PAIR)
    )

    nc = bacc.Bacc("TRN2", target_bir_lowering=False, debug=False,
                   num_devices=N_CORES)
    gc_dram = nc.dram_tensor("gc_in", [128, TOT], dt_data,
                             kind="ExternalInput").ap()
    out_dram = nc.dram_tensor("sino_out", [1, NSLOT * BATCH * NUM_DET],
                              mybir.dt.float32, kind="ExternalOutput").ap()

    nbj = BATCH * NUM_DET
    gc_bufs = 2
    p_bufs = 2 if USE_F32 else 3
    with TileContext(nc) as tc:
        with tc.tile_pool(name="const", bufs=1) as cpool, \
             tc.tile_pool(name="gcpool", bufs=gc_bufs) as gc_pool, \
             tc.tile_pool(name="work", bufs=p_bufs) as pool, \
             tc.tile_pool(name="psum", bufs=2, space="PSUM") as psum_pool:
            ones = cpool.tile([128, 1], dt_data)
            nc.vector.memset(ones[:], 1.0)

            # slots are DMA'd in pairs (one big transfer per 2 slots)
            pair_of = {}

            def _slot_loop():
                for s in range(NSLOT):
                    Rs = int(r_slot[s])
                    fg = g_sizes[s]
                    fc = c_sizes[s]
                    if s % PAIR == 0:
                        members = list(range(s, min(s + PAIR, NSLOT)))
                        plen = sum(g_sizes[t] + c_sizes[t] for t in members)
                        pt = gc_pool.tile([128, PAIRMAX], dt_data, tag="gc")
                        nc.sync.dma_start(
                            out=pt[:, :plen],
                            in_=gc_dram[:, gc_off[s] : gc_off[s] + plen],
                        )
                        base0 = 0
                        for t in members:
                            pair_of[t] = (pt, base0)
                            base0 += g_sizes[t] + c_sizes[t]
                    gc_t, base = pair_of[s]
                    g_t = gc_t[:, base : base + fg]
                    c_t = gc_t[:, base + fg : base + fg + fc]
                    p_t = pool.tile([128, GCMAX], dt_data, tag="p")
                    # c broadcast over b: step-0 must be the OUTERMOST free dim
                    cb = c_t.unsqueeze(1).to_broadcast([128, BATCH, fc])
                    g3 = g_t.rearrange("p (b f) -> p b f", b=BATCH, f=fc)
                    p3 = p_t[:, :fg].rearrange("p (b f) -> p b f", b=BATCH, f=fc)
                    nc.vector.tensor_mul(out=p3, in0=cb, in1=g3)
                    p4 = p_t[:, :fg].rearrange(
                        "p (b r h j) -> p b r h j", b=BATCH, r=Rs, h=2, j=NUM_DET
                    )
                    ps = psum_pool.tile([1, nbj], mybir.dt.float32, space="PSUM")
                    ps2 = ps.rearrange("o (b j) -> o b j", b=BATCH, j=NUM_DET)
                    for b in range(BATCH):
                        for r in range(Rs):
                            for h in range(2):
                                nc.tensor.matmul(
                                    out=ps2[:, b],
                                    lhsT=ones[:],
                                    rhs=p4[:, b, r, h],
                                    start=(r == 0 and h == 0),
                                    stop=(r == Rs - 1 and h == 1),
                                )
                    st = pool.tile([1, nbj], mybir.dt.float32, tag="st")
                    nc.scalar.copy(out=st[:], in_=ps[:])
                    nc.scalar.dma_start(
                        out=out_dram[:, s * nbj : (s + 1) * nbj], in_=st[:]
                    )

            if LOOP > 1:
                with tc.For_i(0, LOOP, 1):
                    _slot_loop()
            else:
                for rep in range(REPEAT):
                    _slot_loop()

    nc.finalize()
    _PROG[key] = (nc, gc_off, g_sizes, c_sizes, TOT)
    return _PROG[key]


# ----------------------------------------------------------------------------
# entry point
# ----------------------------------------------------------------------------

def _host_pack(img: np.ndarray):
    """img [4,1,256,256] f32 -> per-core packed GC [128, TOT] arrays."""
    axes, r_eff, fidx_all, cdev_all, slot_angle, r_slot = _get_tables()
    _, gc_off, g_sizes, c_sizes, TOT = _build_program(0)

    im = img[:, 0].astype(np.float32)
    pad0 = np.zeros((BATCH, IMAGE_SIZE, WPAD), dtype=np.float32)
    pad0[:, :, PAD : PAD + IMAGE_SIZE] = im
    pad1 = np.zeros((BATCH, IMAGE_SIZE, WPAD), dtype=np.float32)
    pad1[:, :, PAD : PAD + IMAGE_SIZE] = im.transpose(0, 2, 1)
    flat = [pad0.reshape(BATCH, -1), pad1.reshape(BATCH, -1)]

    gc_cores = [np.zeros((128, TOT), dtype=_DT_NP) for _ in range(N_CORES)]
    for s in range(NSLOT):
        Rs = int(r_slot[s])
        off = gc_off[s]
        fg = g_sizes[s]
        for k in range(N_CORES):
            a = slot_angle[s, k]
            if a < 0:
                continue
            Ra = int(r_eff[a])
            g = flat[axes[a]][:, fidx_all[a].ravel()]  # [4, Ra*256*256]
            g = g.reshape(BATCH, Ra, 2, 128, NUM_DET)
            # device layout [pl, (b, r, h, j)], r padded to Rs
            gd = np.zeros((128, BATCH, Rs, 2, NUM_DET), dtype=np.float32)
            gd[:, :, :Ra] = g.transpose(3, 0, 1, 2, 4)
            gc_cores[k][:, off : off + fg] = gd.reshape(128, -1).astype(_DT_NP)
            cd = np.zeros((128, Rs, 2, NUM_DET), dtype=_DT_NP)
            cd[:, :Ra] = cdev_all[a].reshape(128, Ra, 2, NUM_DET)
            gc_cores[k][:, off + fg : off + fg + c_sizes[s]] = cd.reshape(
                128, -1
            )
    return gc_cores


def kernel(image: np.ndarray, _trace: bool = False):
    from concourse import bass_utils

    image = np.asarray(image)
    nc = _build_program(0)[0]
    axes, r_eff, fidx_all, cdev_all, slot_angle, r_slot = _get_tables()
    gc_cores = _host_pack(image)

    in_maps = [{"gc_in": gc_cores[k]} for k in range(N_CORES)]

    res = bass_utils.run_bass_kernel_spmd(
        nc, in_maps, core_ids=list(range(N_CORES)), trace=_trace
    )

    sino = np.zeros((BATCH, 1, NUM_ANGLES, NUM_DET), dtype=np.float32)
    for k in range(N_CORES):
        o = res.results[k]["sino_out"].reshape(NSLOT, BATCH, NUM_DET)
        for s in range(NSLOT):
            a = slot_angle[s, k]
            if a >= 0:
                sino[:, 0, a, :] = o[s]
    if _trace:
        return sino, res
    return sino

